# revision 13
# baseline (speedup 1.0000x reference)
"""MAMConv1d Trainium2 kernel.

Y[b,o,l] = max_{c,k}(W[o,c,k] * x[b,c,l+k]) + min_{c,k}(...) + bias[o]
B=8, C=64, L=1024, O=64, K=3, stride=1, Lout=1022.

Strategy (8 NeuronCores, data-parallel over batch B):
- Pair trick: for each c-pair (2cp, 2cp+1) the TensorEngine emits
  s' = (p0+p1)/2 and d' = (p0-p1)/2 directly (block-diagonal weights with
  two entries per column, pre-halved; the k-shift is absorbed into the
  lhsT column offset). ScalarE casts s-chunks with Copy and d-chunks with
  Abs, so   hi = s' + |d'| = max(p0,p1),  lo = s' - |d'| = min(p0,p1)
  are plain add/subtract — which GpSimd's ALU supports (its ISA has no
  min/max). That moves the first (largest) reduction level partially off
  the saturated DVE and shortens both c-trees by one level.
- DVE then runs k-combine max/min + c-pair trees at the 2x fp16 rate.
- bias is folded in on the host after the gather.
- Output is written l-major [1024, 64] per core; host transposes/gathers.
"""

import numpy as np

_B, _C, _L = 8, 64, 1024
_O, _K = 64, 3
_LOUT = (_L - _K) + 1  # 1022
_LPAD = _L + 8  # zero-padded x columns so every matmul window is full
_NT = 8  # l-tiles of 128
_NCP = _C // 2  # 32 c-pairs
_GC = _O * _NCP  # 2048 columns per (k, s/d) chunk

_cache = {}

# Tiles whose hi (k=0,1) level-1 adds run on GpSimd instead of DVE.
_POOL_TILES = frozenset({1, 2, 3, 4, 5, 6})
# Tiles whose casts/level-1 are split in 1024-col halves so the pipeline
# fills faster (PE starts cold at the low p-state).
_SPLIT_TILES = frozenset({0, 1})


def _build_module():
    import concourse.bacc as bacc
    import concourse.bass as bass
    import concourse.mybir as mybir
    import concourse.tile as tile

    f32 = mybir.dt.float32
    f16 = mybir.dt.float16
    nc = bacc.Bacc("TRN2", target_bir_lowering=False, debug=False)

    x_d = nc.dram_tensor("x", [_C, _LPAD], f16, kind="ExternalInput")
    wdp_d = nc.dram_tensor("wdp", [_K, 2, _C, _GC], f16, kind="ExternalInput")
    yt_d = nc.dram_tensor("yt", [_NT * 128, _O], f32, kind="ExternalOutput")

    mx, mn = mybir.AluOpType.max, mybir.AluOpType.min
    ad, sb = mybir.AluOpType.add, mybir.AluOpType.subtract
    X = mybir.AxisListType.X
    Abs = mybir.ActivationFunctionType.Abs

    with tile.TileContext(nc) as tc:
        with (
            tc.tile_pool(name="const", bufs=1) as cpool,
            tc.tile_pool(name="psum", bufs=2, space=bass.MemorySpace.PSUM) as ppool,
            tc.tile_pool(name="outp", bufs=3) as opool,
        ):
            # Startup: tile 0 needs xs[:, 0:130] and the (k0,s)/(k0,d)
            # first halves; issue those first, x on the idle ACT queue.
            xs = cpool.tile([_C, _LPAD], f16)
            wdp = [
                [cpool.tile([_C, _GC], f16, name=f"wdp{k}{sd}") for sd in range(2)]
                for k in range(_K)
            ]
            nc.scalar.dma_start(xs[:, 0:132], x_d[:, 0:132])
            nc.sync.dma_start(wdp[0][0][:, 0:1024], wdp_d[0, 0][:, 0:1024])
            nc.sync.dma_start(wdp[0][1][:, 0:1024], wdp_d[0, 1][:, 0:1024])
            for a, b in [(132, 432), (432, 732), (732, _LPAD)]:
                nc.scalar.dma_start(xs[:, a:b], x_d[:, a:b])
            nc.sync.dma_start(wdp[0][0][:, 1024:2048], wdp_d[0, 0][:, 1024:2048])
            nc.sync.dma_start(wdp[0][1][:, 1024:2048], wdp_d[0, 1][:, 1024:2048])
            for k in range(1, _K):
                for sd in range(2):
                    for a, b in [(0, 1024), (1024, 2048)]:
                        nc.sync.dma_start(wdp[k][sd][:, a:b], wdp_d[k, sd][:, a:b])

            for t in range(_NT):
                s = 128 * t
                split = t in _SPLIT_TILES
                halves = [(0, 1024), (1024, 2048)] if split else [(0, 2048)]
                S = opool.tile([128, _K, 2, _GC], f16, tag="S", bufs=2)
                HI = opool.tile([128, _K, _GC], f16, tag="HI", bufs=2)
                LO = opool.tile([128, _K, _GC], f16, tag="LO", bufs=2)
                for k in range(_K):
                    for sd in range(2):
                        P = ppool.tile([128, _GC], f32, tag="P")
                        for j in range(4):  # one matmul per PSUM bank
                            nc.tensor.matmul(
                                P[:, j * 512 : (j + 1) * 512],
                                xs[:, s + k : s + k + 128],
                                wdp[k][sd][:, j * 512 : (j + 1) * 512],
                            )
                        for a, b in halves:
                            if sd == 0:
                                nc.scalar.copy(S[:, k, 0, a:b], P[:, a:b])
                            else:
                                nc.scalar.activation(S[:, k, 1, a:b], P[:, a:b], Abs)
                # level-1: hi = s' + |d'| (max of pair), lo = s' - |d'|
                use_pool = t in _POOL_TILES
                for a, b in halves:
                    for k in range(_K):
                        eng = nc.gpsimd if (use_pool and k < 2) else nc.vector
                        eng.tensor_tensor(
                            HI[:, k, a:b], S[:, k, 0, a:b], S[:, k, 1, a:b], op=ad
                        )
                    nc.vector.tensor_tensor(
                        LO[:, :, a:b], S[:, :, 0, a:b], S[:, :, 1, a:b], op=sb
                    )
                # k-combine
                HX = opool.tile([128, _GC], f16, tag="HX", bufs=2)
                LN = opool.tile([128, _GC], f16, tag="LN", bufs=2)
                nc.vector.tensor_tensor(HX[:], HI[:, 0, :], HI[:, 1, :], op=mx)
                nc.vector.tensor_tensor(HX[:], HX[:], HI[:, 2, :], op=mx)
                nc.vector.tensor_tensor(LN[:], LO[:, 0, :], LO[:, 1, :], op=mn)
                nc.vector.tensor_tensor(LN[:], LN[:], LO[:, 2, :], op=mn)
                # c-pair trees: 32 -> 16 -> 8 -> 4, then reduce
                hv = HX.rearrange("p (g c) -> p g c", c=_NCP)
                lv = LN.rearrange("p (g c) -> p g c", c=_NCP)
                ux = opool.tile([128, _O, 16], f16, tag="ux")
                un = opool.tile([128, _O, 16], f16, tag="un")
                nc.vector.tensor_tensor(ux[:], hv[:, :, 0:16], hv[:, :, 16:32], op=mx)
                nc.vector.tensor_tensor(un[:], lv[:, :, 0:16], lv[:, :, 16:32], op=mn)
                vx = opool.tile([128, _O, 8], f16, tag="vx")
                vn = opool.tile([128, _O, 8], f16, tag="vn")
                nc.vector.tensor_tensor(vx[:], ux[:, :, 0:8], ux[:, :, 8:16], op=mx)
                nc.vector.tensor_tensor(vn[:], un[:, :, 0:8], un[:, :, 8:16], op=mn)
                wx = opool.tile([128, _O, 4], f16, tag="wx")
                wn = opool.tile([128, _O, 4], f16, tag="wn")
                nc.vector.tensor_tensor(wx[:], vx[:, :, 0:4], vx[:, :, 4:8], op=mx)
                nc.vector.tensor_tensor(wn[:], vn[:, :, 0:4], vn[:, :, 4:8], op=mn)
                ymax = opool.tile([128, _O], f32, tag="ymax")
                ymin = opool.tile([128, _O], f32, tag="ymin")
                nc.vector.tensor_reduce(ymax[:], wx[:], axis=X, op=mx)
                nc.vector.tensor_reduce(ymin[:], wn[:], axis=X, op=mn)
                ysum = opool.tile([128, _O], f32, tag="ysum")
                nc.vector.tensor_tensor(ysum[:], ymax[:], ymin[:], op=ad)
                nc.sync.dma_start(yt_d[s : s + 128, :], ysum[:])

    nc.compile()
    return nc


def _get_module():
    if "nc" not in _cache:
        _cache["nc"] = _build_module()
    return _cache["nc"]


def _pack_weights(weight):
    # wdp[k, 0, c, o*32+cp]: two entries per column -> s' = (p0+p1)/2
    # wdp[k, 1, c, o*32+cp]: +/- entries          -> d' = (p0-p1)/2
    # fp16 halving of W is exact (power of two scale).
    W = weight.astype(np.float32)  # [O, C, K]
    wdp = np.zeros((_K, 2, _C, _GC), dtype=np.float32)
    o = np.arange(_O)
    for k in range(_K):
        for cp in range(_NCP):
            cols = o * _NCP + cp
            w0 = W[:, 2 * cp, k] * 0.5
            w1 = W[:, 2 * cp + 1, k] * 0.5
            wdp[k, 0, 2 * cp, cols] = w0
            wdp[k, 0, 2 * cp + 1, cols] = w1
            wdp[k, 1, 2 * cp, cols] = w0
            wdp[k, 1, 2 * cp + 1, cols] = -w1
    return wdp.astype(np.float16)


def kernel(x, weight, bias, stride):
    from concourse import bass_utils

    x = np.asarray(x, dtype=np.float32)
    weight = np.asarray(weight, dtype=np.float32)
    bias = np.asarray(bias, dtype=np.float32)
    assert int(stride) == 1
    assert x.shape == (_B, _C, _L) and weight.shape == (_O, _C, _K)

    nc = _get_module()

    wdp = _pack_weights(weight)
    xp = np.zeros((_B, _C, _LPAD), dtype=np.float16)
    xp[:, :, :_L] = x

    in_maps = [{"x": xp[b], "wdp": wdp} for b in range(_B)]
    res = bass_utils.run_bass_kernel_spmd(nc, in_maps, core_ids=list(range(_B)))
    _cache["last_results"] = res

    y = np.empty((_B, _O, _LOUT), dtype=np.float32)
    for b in range(_B):
        y[b] = res.results[b]["yt"][:_LOUT, :].T
    y += bias[None, :, None]
    return y


# revision 15
# speedup vs baseline: 1.0907x; 1.0907x over previous
"""MAMConv1d Trainium2 kernel.

Y[b,o,l] = max_{c,k}(W[o,c,k] * x[b,c,l+k]) + min_{c,k}(...) + bias[o]
B=8, C=64, L=1024, O=64, K=3, stride=1, Lout=1022.

Strategy (8 NeuronCores, data-parallel over batch B):
- Pair trick: for each c-pair (2cp, 2cp+1) the TensorEngine emits
  s' = (p0+p1)/2 and d' = (p0-p1)/2 directly (block-diagonal weights with
  two entries per column, pre-halved; the k-shift is absorbed into the
  lhsT column offset). ScalarE casts s-chunks with Copy and d-chunks with
  Abs, so   hi = s' + |d'| = max(p0,p1),  lo = s' - |d'| = min(p0,p1)
  are plain add/subtract — which GpSimd's ALU supports (its ISA has no
  min/max). That moves the first (largest) reduction level partially off
  the saturated DVE and shortens both c-trees by one level.
- DVE then runs k-combine max/min + c-pair trees at the 2x fp16 rate.
- bias is folded in on the host after the gather.
- Output is written l-major [1024, 64] per core; host transposes/gathers.
"""

import numpy as np

_B, _C, _L = 8, 64, 1024
_O, _K = 64, 3
_LOUT = (_L - _K) + 1  # 1022
_LPAD = _L + 8  # zero-padded x columns so every matmul window is full
_NT = 8  # l-tiles of 128
_NCP = _C // 2  # 32 c-pairs
_GC = _O * _NCP  # 2048 columns per (k, s/d) chunk

_cache = {}

# Tiles whose hi (k=0,1) level-1 adds run on GpSimd instead of DVE.
_POOL_TILES = frozenset({1, 2, 3, 4, 5, 6})
# Tiles whose casts/level-1 are split in 1024-col halves so the pipeline
# fills faster (PE starts cold at the low p-state).
_SPLIT_TILES = frozenset({0, 1})


def _build_module():
    import concourse.bacc as bacc
    import concourse.bass as bass
    import concourse.mybir as mybir
    import concourse.tile as tile

    f32 = mybir.dt.float32
    f16 = mybir.dt.float16
    nc = bacc.Bacc("TRN2", target_bir_lowering=False, debug=False)

    x_d = nc.dram_tensor("x", [_C, _LPAD], f16, kind="ExternalInput")
    wdp_d = nc.dram_tensor("wdp", [_K, 2, _C, _GC], f16, kind="ExternalInput")
    yt_d = nc.dram_tensor("yt", [_NT * 128, _O], f32, kind="ExternalOutput")

    mx, mn = mybir.AluOpType.max, mybir.AluOpType.min
    ad, sb = mybir.AluOpType.add, mybir.AluOpType.subtract
    X = mybir.AxisListType.X
    Abs = mybir.ActivationFunctionType.Abs

    with tile.TileContext(nc) as tc:
        with (
            tc.tile_pool(name="const", bufs=1) as cpool,
            tc.tile_pool(name="psum", bufs=2, space=bass.MemorySpace.PSUM) as ppool,
            tc.tile_pool(name="outp", bufs=3) as opool,
        ):
            # Startup: tile 0 needs xs[:, 0:130] and the (k0,s)/(k0,d)
            # first halves; issue those first, x on the idle ACT queue.
            xs = cpool.tile([_C, _LPAD], f16)
            wdp = [
                [cpool.tile([_C, _GC], f16, name=f"wdp{k}{sd}") for sd in range(2)]
                for k in range(_K)
            ]
            nc.scalar.dma_start(xs[:, 0:132], x_d[:, 0:132])
            nc.sync.dma_start(wdp[0][0][:, 0:1024], wdp_d[0, 0][:, 0:1024])
            nc.sync.dma_start(wdp[0][1][:, 0:1024], wdp_d[0, 1][:, 0:1024])
            for a, b in [(132, 432), (432, 732), (732, _LPAD)]:
                nc.scalar.dma_start(xs[:, a:b], x_d[:, a:b])
            nc.sync.dma_start(wdp[0][0][:, 1024:2048], wdp_d[0, 0][:, 1024:2048])
            nc.sync.dma_start(wdp[0][1][:, 1024:2048], wdp_d[0, 1][:, 1024:2048])
            for k in range(1, _K):
                for sd in range(2):
                    for a, b in [(0, 1024), (1024, 2048)]:
                        nc.sync.dma_start(wdp[k][sd][:, a:b], wdp_d[k, sd][:, a:b])

            for t in range(_NT):
                s = 128 * t
                split = t in _SPLIT_TILES
                halves = [(0, 1024), (1024, 2048)] if split else [(0, 2048)]
                S = opool.tile([128, _K, 2, _GC], f16, tag="S", bufs=2)
                HI = opool.tile([128, _K, _GC], f16, tag="HI", bufs=2)
                LO = opool.tile([128, _K, _GC], f16, tag="LO", bufs=2)
                for k in range(_K):
                    for sd in range(2):
                        P = ppool.tile([128, _GC], f32, tag="P")
                        for j in range(4):  # one matmul per PSUM bank
                            nc.tensor.matmul(
                                P[:, j * 512 : (j + 1) * 512],
                                xs[:, s + k : s + k + 128],
                                wdp[k][sd][:, j * 512 : (j + 1) * 512],
                            )
                        for a, b in halves:
                            if sd == 0:
                                nc.scalar.copy(S[:, k, 0, a:b], P[:, a:b])
                            else:
                                nc.scalar.activation(S[:, k, 1, a:b], P[:, a:b], Abs)
                # level-1: hi = s' + |d'| (max of pair), lo = s' - |d'|
                # Pool absorbs one 2048-col add per tile (its throughput is
                # ~0.5 elem/ns — more than that and it paces the chain).
                use_pool = t in _POOL_TILES
                for a, b in halves:
                    eng = nc.gpsimd if use_pool else nc.vector
                    eng.tensor_tensor(
                        HI[:, 0, a:b], S[:, 0, 0, a:b], S[:, 0, 1, a:b], op=ad
                    )
                    nc.vector.tensor_tensor(
                        HI[:, 1:3, a:b], S[:, 1:3, 0, a:b], S[:, 1:3, 1, a:b], op=ad
                    )
                    nc.vector.tensor_tensor(
                        LO[:, :, a:b], S[:, :, 0, a:b], S[:, :, 1, a:b], op=sb
                    )
                # k-combine
                HX = opool.tile([128, _GC], f16, tag="HX", bufs=2)
                LN = opool.tile([128, _GC], f16, tag="LN", bufs=2)
                nc.vector.tensor_tensor(HX[:], HI[:, 0, :], HI[:, 1, :], op=mx)
                nc.vector.tensor_tensor(HX[:], HX[:], HI[:, 2, :], op=mx)
                nc.vector.tensor_tensor(LN[:], LO[:, 0, :], LO[:, 1, :], op=mn)
                nc.vector.tensor_tensor(LN[:], LN[:], LO[:, 2, :], op=mn)
                # c-pair trees: 32 -> 16 -> 8 -> 4, then reduce
                hv = HX.rearrange("p (g c) -> p g c", c=_NCP)
                lv = LN.rearrange("p (g c) -> p g c", c=_NCP)
                ux = opool.tile([128, _O, 16], f16, tag="ux")
                un = opool.tile([128, _O, 16], f16, tag="un")
                nc.vector.tensor_tensor(ux[:], hv[:, :, 0:16], hv[:, :, 16:32], op=mx)
                nc.vector.tensor_tensor(un[:], lv[:, :, 0:16], lv[:, :, 16:32], op=mn)
                vx = opool.tile([128, _O, 8], f16, tag="vx")
                vn = opool.tile([128, _O, 8], f16, tag="vn")
                nc.vector.tensor_tensor(vx[:], ux[:, :, 0:8], ux[:, :, 8:16], op=mx)
                nc.vector.tensor_tensor(vn[:], un[:, :, 0:8], un[:, :, 8:16], op=mn)
                wx = opool.tile([128, _O, 4], f16, tag="wx")
                wn = opool.tile([128, _O, 4], f16, tag="wn")
                nc.vector.tensor_tensor(wx[:], vx[:, :, 0:4], vx[:, :, 4:8], op=mx)
                nc.vector.tensor_tensor(wn[:], vn[:, :, 0:4], vn[:, :, 4:8], op=mn)
                ymax = opool.tile([128, _O], f32, tag="ymax")
                ymin = opool.tile([128, _O], f32, tag="ymin")
                nc.vector.tensor_reduce(ymax[:], wx[:], axis=X, op=mx)
                nc.vector.tensor_reduce(ymin[:], wn[:], axis=X, op=mn)
                ysum = opool.tile([128, _O], f32, tag="ysum")
                eng = nc.gpsimd if t in _POOL_TILES else nc.vector
                eng.tensor_tensor(ysum[:], ymax[:], ymin[:], op=ad)
                nc.sync.dma_start(yt_d[s : s + 128, :], ysum[:])

    nc.compile()
    return nc


def _get_module():
    if "nc" not in _cache:
        _cache["nc"] = _build_module()
    return _cache["nc"]


def _pack_weights(weight):
    # wdp[k, 0, c, o*32+cp]: two entries per column -> s' = (p0+p1)/2
    # wdp[k, 1, c, o*32+cp]: +/- entries          -> d' = (p0-p1)/2
    # fp16 halving of W is exact (power of two scale).
    W = weight.astype(np.float32)  # [O, C, K]
    wdp = np.zeros((_K, 2, _C, _GC), dtype=np.float32)
    o = np.arange(_O)
    for k in range(_K):
        for cp in range(_NCP):
            cols = o * _NCP + cp
            w0 = W[:, 2 * cp, k] * 0.5
            w1 = W[:, 2 * cp + 1, k] * 0.5
            wdp[k, 0, 2 * cp, cols] = w0
            wdp[k, 0, 2 * cp + 1, cols] = w1
            wdp[k, 1, 2 * cp, cols] = w0
            wdp[k, 1, 2 * cp + 1, cols] = -w1
    return wdp.astype(np.float16)


def kernel(x, weight, bias, stride):
    from concourse import bass_utils

    x = np.asarray(x, dtype=np.float32)
    weight = np.asarray(weight, dtype=np.float32)
    bias = np.asarray(bias, dtype=np.float32)
    assert int(stride) == 1
    assert x.shape == (_B, _C, _L) and weight.shape == (_O, _C, _K)

    nc = _get_module()

    wdp = _pack_weights(weight)
    xp = np.zeros((_B, _C, _LPAD), dtype=np.float16)
    xp[:, :, :_L] = x

    in_maps = [{"x": xp[b], "wdp": wdp} for b in range(_B)]
    res = bass_utils.run_bass_kernel_spmd(nc, in_maps, core_ids=list(range(_B)))
    _cache["last_results"] = res

    y = np.empty((_B, _O, _LOUT), dtype=np.float32)
    for b in range(_B):
        y[b] = res.results[b]["yt"][:_LOUT, :].T
    y += bias[None, :, None]
    return y
